# revision 2
# baseline (speedup 1.0000x reference)
"""Bilateral filter (K=7, guide channels=3) Trainium2 Bass kernel, v3.

v1's robust two-phase schedule with a compressed guide phase:
  Phase 1 (~40us): 25 guide maps in fp16, whole chain on DVE
    (sub 470ns, square-as-self-mult 470ns, two 262-wide adds 394ns)
    with exp + remap-DMA issue on ACT. Map 0 is memset(1.0).
  Phase 2 (~90us): 49 apply products; 41 on DVE (2.2us), 8 on GPSIMD
    (8.3us) whose identity-matmul groups are deferred in the PE stream
    (one after every 4th DVE product) so the PE order never couples the
    fast stream to the slow one. Norm = GPSIMD add-tree over KA during
    its idle tail. Odd-parity input copies come straight from DRAM.

Math identical to v1 (see kernel.py).
"""

import numpy as np

B, C, H, W = 2, 32, 256, 256
CG = 3
R = 3
NB = 4
RB = H // NB               # 64 out rows per core
NCORES = 8

GR = RB + 2 * R            # 70
GXH = 276                  # host guide slab cols -10..265
GX2 = 270                  # G4/G4O slab width (cols -7..262 / -6..263)
AL0 = 4                    # aligned (-3) col offset within a G4 slab
IX = W + 2 * R             # 262
MR = RB + R                # 67
MJ = W + 2 * R             # 262
MS = W + 2 * R             # 262
WH = 2
XC = W // WH               # 128
XW = XC + 2 * R            # 134
NSLOT = 5
SRW = 2 * MJ

UPLUS = [(0, ux) for ux in range(0, R + 1)] + [
    (uy, ux) for uy in range(1, R + 1) for ux in range(-R, R + 1)
]
ALL_U = []
for uy in range(-R, R + 1):
    for ux in range(-R, R + 1):
        if (uy, ux) in UPLUS:
            ALL_U.append((uy, ux, UPLUS.index((uy, ux)), False))
        else:
            ALL_U.append((uy, ux, UPLUS.index((-uy, -ux)), True))

_COMPILED = None


def _build_nc(legalize=True):
    import concourse.bass as bass
    import concourse.mybir as mybir
    from concourse.bass import AP
    from concourse.tile import TileContext, add_dep_helper

    fp32 = mybir.dt.float32
    fp16 = mybir.dt.float16
    ALU = mybir.AluOpType
    ACTF = mybir.ActivationFunctionType

    nc = bass.Bass()

    guide_d = nc.declare_dram_parameter("guide", [CG, GR, GXH], fp32, isOutput=False)
    inp_d = nc.declare_dram_parameter("inp", [C, GR, IX], fp32, isOutput=False)
    rr_d = nc.declare_dram_parameter("rr25", [1, 25], fp32, isOutput=False)
    sig_d = nc.declare_dram_parameter("sigma", [1, 1], fp32, isOutput=False)
    ident_d = nc.declare_dram_parameter("ident", [128, 128], fp16, isOutput=False)
    out_d = nc.declare_dram_parameter("out", [C, RB, W], fp32, isOutput=True)

    def sb(t, p0, pn, off, dims):
        sl = t[p0:p0 + pn]
        return AP(sl.tensor, sl.offset + off, [sl.ap[0], *dims])

    def dr_ap(d, off, dims):
        full = d[:]
        return AP(full.tensor, full.offset + off, dims)

    from contextlib import ExitStack

    with TileContext(nc) as tc, ExitStack() as es:
        _base0 = ((nc.sbuf_base + 31) // 32) * 32
        _ARENA_BYTES = 207 * 1024
        es.enter_context(nc.sbuf_tensor("ARENA", [128, _ARENA_BYTES], mybir.dt.uint8))
        _off = [_base0]

        def at(name, shape, dt, offset=None):
            if offset is None:
                offset = _off[0]
            import functools, operator
            sz = functools.reduce(operator.mul, shape[1:]) * mybir.dt.size(dt)
            h = nc.alloc_sbuf_tensor_at(name, shape, dt, offset=offset, align_bytes=32)
            _off[0] = max(_off[0], offset + ((sz + 31) // 32) * 32)
            return h

        INB7 = at("INB7", [128, C * 7 * XW], fp16)
        INB7O = at("INB7O", [128, C * 7 * XW], fp16)
        _d3r_off = _off[0]
        D3R = at("D3R", [128, NSLOT * CG * MJ], fp16)
        _sqr_off = _off[0]
        SQR = at("SQR", [128, NSLOT * CG * MJ], fp16)
        SRR = at("SRR", [128, NSLOT * SRW], fp16)
        _g4_off = _off[0]
        G4 = at("G4", [128, 4 * CG * GX2], fp16)
        G4O = at("G4O", [128, 4 * CG * GX2], fp16)
        _k25_off = _off[0]
        K25 = at("K25", [128, 25 * MS], fp16)
        KA = at("KA", [128, 49 * XC], fp16)
        _pbuf_off = _off[0]
        PBUFS = [at(f"P{i}T", [128, 4096], fp16) for i in range(3)]
        NORM = at("NORM", [128, XC], fp32)
        RCP = at("RCP", [128, XC], fp32)
        BROWT = at("BROWT", [128, 25], fp32)
        IDENT = at("IDENT", [128, 128], fp16)
        BROW = at("BROW", [1, 25], fp32)
        SIG = at("SIG", [1, 1], fp32)
        SIG2 = at("SIG2", [1, 1], fp32)
        IS2 = at("IS2", [1, 1], fp32)
        RR = at("RR", [1, 25], fp32)
        assert _off[0] <= _base0 + _ARENA_BYTES, (_off[0] - _base0,)
        # phase-2 aliases over phase-1-only tensors
        T1 = nc.alloc_sbuf_tensor_at("T1", [128, 24 * XC], fp16,
                                     offset=_d3r_off, align_bytes=32)
        OUTC = nc.alloc_sbuf_tensor_at("OUTC", [128, 2 * 8 * XC], fp32,
                                       offset=_pbuf_off, align_bytes=32)
        PBUF3 = nc.alloc_sbuf_tensor_at("P3T", [128, 4096], fp16,
                                        offset=_g4_off, align_bytes=32)
        PBUF5 = nc.alloc_sbuf_tensor_at("P5T", [128, 4096], fp16,
                                        offset=_sqr_off, align_bytes=32)
        PBUF6 = nc.alloc_sbuf_tensor_at("P6T", [128, 4096], fp16,
                                        offset=_k25_off, align_bytes=32)
        ACC = es.enter_context(nc.psum_tensor("ACC", [128, 4096], fp32))

        v, s, g, t, sync = nc.vector, nc.scalar, nc.gpsimd, nc.tensor, nc.sync

        # ---- constants (emitted after the first guide loads; see below) ----
        def _emit_consts():
            sync.dma_start(out=RR[:], in_=rr_d[:])
            sync.dma_start(out=SIG[:], in_=sig_d[:])
            v.tensor_tensor(out=SIG2[:], in0=SIG[:], in1=SIG[:], op=ALU.mult)
            v.reciprocal(out=IS2[:], in_=SIG2[:])
            v.tensor_scalar(out=BROW[:], in0=RR[:], scalar1=IS2[0:1, 0:1],
                            scalar2=-0.5, op0=ALU.mult, op1=ALU.mult)
            wscr = nc.dram_tensor("wscr", [1, 25], fp32)
            sync.dma_start(out=wscr[:], in_=BROW[:])
            sync.dma_start(out=BROWT[:],
                           in_=dr_ap(wscr, 0, [[0, 128], [1, 25]]))

        # ---- fp16 staging casts (software-DGE d2d on gpsimd queue) ----
        GHD = nc.dram_tensor("GHD", [1, CG * GR * GXH + 2], fp16)
        g.dma_start(out=dr_ap(GHD, 1, [[1, CG * GR * GXH]]),
                    in_=dr_ap(guide_d, 0, [[1, CG * GR * GXH]]))
        INHD = nc.dram_tensor("INHD", [1, C * GR * IX + 2], fp16)
        _csz = (C // 4) * GR * IX
        for cc in range(4):
            g.dma_start(out=dr_ap(INHD, 1 + cc * _csz, [[1, _csz]]),
                        in_=dr_ap(inp_d, cc * _csz, [[1, _csz]]))

        # ---- guide slabs, dy0 pair first ----
        def g4_pair(dy):
            dst = sb(G4, 0, MR, dy * (CG * GX2), [[GX2, CG], [1, GX2]])
            src = dr_ap(GHD, 1 + dy * GXH + 3, [[GXH, MR], [GR * GXH, CG], [1, GX2]])
            sync.dma_start(out=dst, in_=src)
            dst = sb(G4O, 0, MR, dy * (CG * GX2), [[GX2, CG], [1, GX2]])
            src = dr_ap(GHD, 1 + dy * GXH + 4, [[GXH, MR], [GR * GXH, CG], [1, GX2]])
            sync.dma_start(out=dst, in_=src)

        DR_ORDER = [3, 4, 2, 5, 1, 6, 0]

        def inb7_load(j):
            dr = DR_ORDER[j]
            for wh in range(WH):
                dst = sb(INB7, wh * 64, 64, dr * XW, [[7 * XW, C], [1, XW]])
                src = dr_ap(INHD, 1 + dr * IX + wh * XC,
                            [[IX, 64], [GR * IX, C], [1, XW]])
                sync.dma_start(out=dst, in_=src)

        inb7o_dmas = []

        def inb7o_load(j):
            dr = DR_ORDER[j]
            for wh in range(WH):
                dst = sb(INB7O, wh * 64, 64, dr * XW + 1,
                         [[7 * XW, C], [1, XW - 1]])
                src = dr_ap(INHD, 1 + dr * IX + wh * XC,
                            [[IX, 64], [GR * IX, C], [1, XW - 1]])
                inb7o_dmas.append(sync.dma_start(out=dst, in_=src))

        g4_pair(0)
        _emit_consts()
        inb7_load(0)
        for _dy in (1, 2, 3):
            g4_pair(_dy)
        sync.dma_start(out=IDENT[:], in_=ident_d[:])
        for _j in range(1, 7):
            inb7_load(_j)
        for _j in range(7):
            inb7o_load(_j)

        # ---- kA slots and remap groups (all issued during phase 1) ----
        ka_slot = {}
        for (uy, ux, m, shifted) in ALL_U:
            ka_slot[(uy, ux)] = m if not shifted else 24 + m
        AL_CHUNKS = {}
        CH = 4
        for m0c in range(0, 25, CH):
            nmap = min(CH, 25 - m0c)
            AL_CHUNKS.setdefault(m0c + nmap - 1, []).append((m0c, nmap))
        SH_GROUPS = {}
        _vy_base = {0: 1, 1: 4, 2: 11, 3: 18}
        for (vy, mb, nmap) in [(0, 1, 3), (1, 4, 4), (1, 8, 3), (2, 11, 4),
                               (2, 15, 3), (3, 18, 4), (3, 22, 3)]:
            vxmin = ((-3 if vy >= 1 else 1) + (mb - _vy_base[vy]))
            SH_GROUPS.setdefault(mb + nmap - 1, []).append((vy, mb, nmap, vxmin))

        # ---- phase 1: guide maps, DVE chain + ACT exp/remap-issue ----
        HOLD_MAP = 12
        last_exp = None
        for m, (uy, ux) in enumerate(UPLUS):
            if m == 0:
                g.memset(sb(K25, 0, MR, 0, [[1, MJ]]), 1.0)
            else:
                sl = m % NSLOT
                base = uy * (CG * GX2)
                if ux % 2 == 0:
                    in0 = sb(G4, 0, MR, base + AL0 + ux, [[GX2, CG], [1, MJ]])
                else:
                    in0 = sb(G4O, 0, MR, base + 3 + ux, [[GX2, CG], [1, MJ]])
                in1 = sb(G4, 0, MR, AL0, [[GX2, CG], [1, MJ]])
                v.tensor_tensor(out=sb(D3R, 0, MR, sl * CG * MJ,
                                       [[MJ, CG], [1, MJ]]),
                                in0=in0, in1=in1, op=ALU.subtract)
                v.tensor_tensor(out=sb(SQR, 0, MR, sl * CG * MJ, [[1, CG * MJ]]),
                                in0=sb(D3R, 0, MR, sl * CG * MJ, [[1, CG * MJ]]),
                                in1=sb(D3R, 0, MR, sl * CG * MJ, [[1, CG * MJ]]),
                                op=ALU.mult)
                v.tensor_tensor(out=sb(SRR, 0, MR, sl * SRW, [[1, MJ]]),
                                in0=sb(SQR, 0, MR, sl * CG * MJ, [[1, MJ]]),
                                in1=sb(SQR, 0, MR, sl * CG * MJ + MJ, [[1, MJ]]),
                                op=ALU.add)
                v.tensor_tensor(out=sb(SRR, 0, MR, sl * SRW + MJ, [[1, MJ]]),
                                in0=sb(SRR, 0, MR, sl * SRW, [[1, MJ]]),
                                in1=sb(SQR, 0, MR, sl * CG * MJ + 2 * MJ,
                                       [[1, MJ]]),
                                op=ALU.add)
                _e = s.activation(out=sb(K25, 0, MR, m * MS, [[1, MJ]]),
                                  in_=sb(SRR, 0, MR, sl * SRW + MJ,
                                         [[1, MJ]]),
                                  func=ACTF.Exp, scale=-0.5,
                                  bias=BROWT[0:MR, m:m + 1])
                if m == HOLD_MAP:
                    last_exp = _e
                if m == 18:
                    gate_exp = _e
            for (m0c, nmap) in AL_CHUNKS.get(m, ()):
                for wh in range(WH):
                    dst = sb(KA, wh * 64, 64, m0c * XC, [[XC, nmap], [1, XC]])
                    srcc = sb(K25, 3, 64, m0c * MS + wh * XC + 3,
                              [[MS, nmap], [1, XC]])
                    s.dma_start(out=dst, in_=srcc)
            for (vy, mb, nmap, vxmin) in SH_GROUPS.get(m, ()):
                for wh in range(WH):
                    dst = sb(KA, wh * 64, 64, (24 + mb) * XC,
                             [[XC, nmap], [1, XC]])
                    srcc = sb(K25, 3 - vy, 64,
                              mb * MS + wh * XC + 3 - vxmin,
                              [[MS - 1, nmap], [1, XC]])
                    s.dma_start(out=dst, in_=srcc)

        # ---- phase 2: products ----
        evens = sorted((e for e in ALL_U if (3 + e[1]) % 2 == 0),
                       key=lambda e: e[2])
        odds = sorted((e for e in ALL_U if (3 + e[1]) % 2 == 1),
                      key=lambda e: e[2])
        order = evens + odds
        POOL_POS = {3, 9, 15, 21, 27, 33, 39, 45}
        n_mm = [0]
        ndve = [0]
        npool = [0]
        dve_since = [0]
        pend_pool = []
        dve_tts = []
        pool_tts = []

        def emit_mms(P):
            first = n_mm[0] == 0
            last = n_mm[0] == len(order) - 1
            for bk in range(8):
                t.matmul(ACC[:, bk * 512:(bk + 1) * 512], IDENT[:, :],
                         P[:, bk * 512:(bk + 1) * 512], start=first, stop=last)
            n_mm[0] += 1

        for oi, ent in enumerate(order):
            uy, ux, m, shifted = ent
            ui = ka_slot[(uy, ux)]
            off = (uy + 3) * XW + 3 + ux
            if (3 + ux) % 2 == 0:
                in0 = sb(INB7, 0, 128, off, [[7 * XW, C], [1, XC]])
            else:
                in0 = sb(INB7O, 0, 128, off + 1, [[7 * XW, C], [1, XC]])
            in1 = sb(KA, 0, 128, ui * XC, [[0, C], [1, XC]])
            if oi in POOL_POS:
                P = PBUF5 if npool[0] % 2 == 0 else PBUF6
                npool[0] += 1
                _pt = g.tensor_tensor(out=sb(P, 0, 128, 0, [[XC, C], [1, XC]]),
                                      in0=in0, in1=in1, op=ALU.mult)
                pool_tts.append(_pt)
                pend_pool.append(P)
            else:
                k = ndve[0]
                ndve[0] += 1
                cyc = [PBUFS[0], PBUFS[1], PBUFS[2], PBUF3]
                P = cyc[k % 4]
                _tt = v.tensor_tensor(out=sb(P, 0, 128, 0, [[XC, C], [1, XC]]),
                                      in0=in0, in1=in1, op=ALU.mult)
                dve_tts.append(_tt)
                if oi < 5 and last_exp is not None:
                    add_dep_helper(_tt.ins, last_exp.ins, sync=False,
                                   reason="hold products until guide done")
                emit_mms(P)
                dve_since[0] += 1
                if pend_pool and dve_since[0] >= 4:
                    emit_mms(pend_pool.pop(0))
                    dve_since[0] = 0
            # norm tree on gpsimd once its products are done
            if oi == 46:
                _nt = g.tensor_tensor(out=sb(T1, 0, 128, 0, [[1, 24 * XC]]),
                                in0=sb(KA, 0, 128, 0, [[1, 24 * XC]]),
                                in1=sb(KA, 0, 128, 24 * XC, [[1, 24 * XC]]),
                                op=ALU.add)
                add_dep_helper(_nt.ins, pool_tts[-1].ins, sync=False,
                               reason="norm tree after pool products")
                for nblk in (12, 6, 3):
                    g.tensor_tensor(out=sb(T1, 0, 128, 0, [[1, nblk * XC]]),
                                    in0=sb(T1, 0, 128, 0, [[1, nblk * XC]]),
                                    in1=sb(T1, 0, 128, nblk * XC,
                                           [[1, nblk * XC]]),
                                    op=ALU.add)
                g.tensor_tensor(out=sb(T1, 0, 128, 0, [[1, XC]]),
                                in0=sb(T1, 0, 128, 0, [[1, XC]]),
                                in1=sb(T1, 0, 128, XC, [[1, XC]]),
                                op=ALU.add)
                g.tensor_tensor(out=sb(T1, 0, 128, 0, [[1, XC]]),
                                in0=sb(T1, 0, 128, 0, [[1, XC]]),
                                in1=sb(T1, 0, 128, 2 * XC, [[1, XC]]),
                                op=ALU.add)
                g.tensor_tensor(out=sb(NORM, 0, 128, 0, [[1, XC]]),
                                in0=sb(T1, 0, 128, 0, [[1, XC]]),
                                in1=sb(KA, 0, 128, 48 * XC, [[1, XC]]),
                                op=ALU.add)
        while pend_pool:
            emit_mms(pend_pool.pop(0))
        _rc = v.reciprocal(out=RCP[:, :], in_=NORM[:, :])
        from concourse.tile import add_dep_helper as _adh
        _adh(_rc.ins, dve_tts[-2].ins, sync=False,
             reason="recip after products (scheduler hint)")

        # ---- finish: out = acc * rcp (bcast over c) ----
        for ch in range(4):
            obuf = (ch % 2) * 8 * XC
            a_sl = ACC[:, ch * 1024:(ch + 1) * 1024]
            a_ap = AP(a_sl.tensor, a_sl.offset, [a_sl.ap[0], [XC, 8], [1, XC]])
            r_ap = sb(RCP, 0, 128, 0, [[0, 8], [1, XC]])
            o_ap = sb(OUTC, 0, 128, obuf, [[XC, 8], [1, XC]])
            v.tensor_tensor(out=o_ap, in0=a_ap, in1=r_ap, op=ALU.mult)
            for wh in range(WH):
                srcc = sb(OUTC, wh * 64, 64, obuf, [[XC, 8], [1, XC]])
                dst = dr_ap(out_d, ch * 8 * RB * W + wh * XC,
                            [[W, 64], [RB * W, 8], [1, XC]])
                sync.dma_start(out=dst, in_=srcc)

    if legalize:
        _legalize_waits(nc)
    return nc


def _legalize_waits(nc):
    import concourse.mybir as mybir

    ctr = [0]
    for bb in nc.main_func.blocks:
        out = []
        changed = False
        for ins in bb.instructions:
            cap = 1
            si = ins.sync_info
            waits = list(si.on_wait) if si is not None else []
            if len(waits) > cap:
                keep = waits[:cap]
                extra = waits[cap:]
                while extra:
                    chunk, extra = extra[:1], extra[1:]
                    e = mybir.InstEventSemaphore(
                        name=f"wsplit-{ctr[0]}", ins=[], outs=[])
                    ctr[0] += 1
                    e.engine = ins.engine
                    e.sync_info = mybir.SyncInfo(on_wait=chunk, on_update=[])
                    out.append(e)
                ins.sync_info = mybir.SyncInfo(on_wait=keep,
                                               on_update=list(si.on_update))
                changed = True
            out.append(ins)
        if changed:
            bb.instructions = out
    return nc


def _host_prep(input, input_for_kernel, sigma_for_kernel):
    inp = np.asarray(input, dtype=np.float32)
    gui = np.asarray(input_for_kernel, dtype=np.float32)
    sig = np.float32(np.asarray(sigma_for_kernel).reshape(()))

    gp = np.zeros((B, CG, H + 12, W + 20), dtype=np.float32)
    gp[:, :, 6:6 + H, 10:10 + W] = gui
    ip = np.zeros((B, C, H + 12, W + 12), dtype=np.float32)
    ip[:, :, 6:6 + H, 6:6 + W] = inp

    rr = np.array([[float(uy * uy + ux * ux) for (uy, ux) in UPLUS]],
                  dtype=np.float32)
    ident = np.eye(128, dtype=np.float16)
    sig_arr = np.array([[sig]], dtype=np.float32)

    in_maps = []
    for core in range(NCORES):
        b, hb = divmod(core, NB)
        r0 = hb * RB
        gs = gp[b, :, 3 + r0: 3 + r0 + GR, 0:GXH]
        is_ = ip[b, :, 3 + r0: 3 + r0 + GR, 3:3 + IX]
        in_maps.append({
            "guide": np.ascontiguousarray(gs),
            "inp": np.ascontiguousarray(is_),
            "rr25": rr,
            "sigma": sig_arr,
            "ident": ident,
        })
    return in_maps


def kernel(input, input_for_kernel, sigma_for_kernel):
    global _COMPILED
    from concourse.bass_utils import run_bass_kernel_spmd

    if _COMPILED is None:
        _COMPILED = _build_nc()
    nc = _COMPILED

    in_maps = _host_prep(input, input_for_kernel, sigma_for_kernel)
    res = run_bass_kernel_spmd(nc, in_maps, core_ids=list(range(NCORES)))
    out = np.zeros((B, C, H, W), dtype=np.float32)
    for core in range(NCORES):
        b, hb = divmod(core, NB)
        out[b, :, hb * RB:(hb + 1) * RB, :] = res.results[core]["out"]
    return out


# revision 3
# speedup vs baseline: 2.6610x; 2.6610x over previous
"""Bilateral filter (K=7, guide channels=3) Trainium2 Bass kernel, v3.

v1's robust two-phase schedule with a compressed guide phase:
  Phase 1 (~40us): 25 guide maps in fp16, whole chain on DVE
    (sub 470ns, square-as-self-mult 470ns, two 262-wide adds 394ns)
    with exp + remap-DMA issue on ACT. Map 0 is memset(1.0).
  Phase 2 (~90us): 49 apply products; 41 on DVE (2.2us), 8 on GPSIMD
    (8.3us) whose identity-matmul groups are deferred in the PE stream
    (one after every 4th DVE product) so the PE order never couples the
    fast stream to the slow one. Norm = GPSIMD add-tree over KA during
    its idle tail. Odd-parity input copies come straight from DRAM.

Math identical to v1 (see kernel.py).
"""

import numpy as np

B, C, H, W = 2, 32, 256, 256
CG = 3
R = 3
NB = 4
RB = H // NB               # 64 out rows per core
NCORES = 8

GR = RB + 2 * R            # 70
GXH = 276                  # host guide slab cols -10..265
GX2 = 270                  # G4/G4O slab width (cols -7..262 / -6..263)
AL0 = 4                    # aligned (-3) col offset within a G4 slab
IX = W + 2 * R             # 262
MR = RB + R                # 67
MJ = W + 2 * R             # 262
MS = W + 2 * R             # 262
WH = 2
XC = W // WH               # 128
XW = XC + 2 * R            # 134
NSLOT = 5
SRW = 2 * MJ

UPLUS = [(0, ux) for ux in range(0, R + 1)] + [
    (uy, ux) for uy in range(1, R + 1) for ux in range(-R, R + 1)
]
ALL_U = []
for uy in range(-R, R + 1):
    for ux in range(-R, R + 1):
        if (uy, ux) in UPLUS:
            ALL_U.append((uy, ux, UPLUS.index((uy, ux)), False))
        else:
            ALL_U.append((uy, ux, UPLUS.index((-uy, -ux)), True))

_COMPILED = None


def _build_nc(legalize=True):
    import concourse.bass as bass
    import concourse.mybir as mybir
    from concourse.bass import AP
    from concourse.tile import TileContext, add_dep_helper

    fp32 = mybir.dt.float32
    fp16 = mybir.dt.float16
    ALU = mybir.AluOpType
    ACTF = mybir.ActivationFunctionType

    nc = bass.Bass()

    guide_d = nc.declare_dram_parameter("guide", [CG, GR, GXH], fp32, isOutput=False)
    inp_d = nc.declare_dram_parameter("inp", [C, GR, IX], fp32, isOutput=False)
    rr_d = nc.declare_dram_parameter("rr25", [1, 25], fp32, isOutput=False)
    sig_d = nc.declare_dram_parameter("sigma", [1, 1], fp32, isOutput=False)
    ident_d = nc.declare_dram_parameter("ident", [128, 128], fp16, isOutput=False)
    out_d = nc.declare_dram_parameter("out", [C, RB, W], fp32, isOutput=True)

    def sb(t, p0, pn, off, dims):
        sl = t[p0:p0 + pn]
        return AP(sl.tensor, sl.offset + off, [sl.ap[0], *dims])

    def dr_ap(d, off, dims):
        full = d[:]
        return AP(full.tensor, full.offset + off, dims)

    from contextlib import ExitStack

    with TileContext(nc) as tc, ExitStack() as es:
        _base0 = ((nc.sbuf_base + 31) // 32) * 32
        _ARENA_BYTES = 207 * 1024
        es.enter_context(nc.sbuf_tensor("ARENA", [128, _ARENA_BYTES], mybir.dt.uint8))
        _off = [_base0]

        def at(name, shape, dt, offset=None):
            if offset is None:
                offset = _off[0]
            import functools, operator
            sz = functools.reduce(operator.mul, shape[1:]) * mybir.dt.size(dt)
            h = nc.alloc_sbuf_tensor_at(name, shape, dt, offset=offset, align_bytes=32)
            _off[0] = max(_off[0], offset + ((sz + 31) // 32) * 32)
            return h

        INB7 = at("INB7", [128, C * 7 * XW], fp16)
        INB7O = at("INB7O", [128, C * 7 * XW], fp16)
        _d3r_off = _off[0]
        D3R = at("D3R", [128, NSLOT * CG * MJ], fp16)
        _sqr_off = _off[0]
        SQR = at("SQR", [128, NSLOT * CG * MJ], fp16)
        SRR = at("SRR", [128, NSLOT * SRW], fp16)
        _g4_off = _off[0]
        G4 = at("G4", [128, 4 * CG * GX2], fp16)
        G4O = at("G4O", [128, 4 * CG * GX2], fp16)
        _k25_off = _off[0]
        K25 = at("K25", [128, 25 * MS], fp16)
        KA = at("KA", [128, 49 * XC], fp16)
        _pbuf_off = _off[0]
        PBUFS = [at(f"P{i}T", [128, 4096], fp16) for i in range(3)]
        NORM = at("NORM", [128, XC], fp32)
        RCP = at("RCP", [128, XC], fp32)
        BROWT = at("BROWT", [128, 25], fp32)
        IDENT = at("IDENT", [128, 128], fp16)
        BROW = at("BROW", [1, 25], fp32)
        SIG = at("SIG", [1, 1], fp32)
        SIG2 = at("SIG2", [1, 1], fp32)
        IS2 = at("IS2", [1, 1], fp32)
        RR = at("RR", [1, 25], fp32)
        assert _off[0] <= _base0 + _ARENA_BYTES, (_off[0] - _base0,)
        # phase-2 aliases over phase-1-only tensors
        T1 = nc.alloc_sbuf_tensor_at("T1", [128, 24 * XC], fp16,
                                     offset=_d3r_off, align_bytes=32)
        OUTC = nc.alloc_sbuf_tensor_at("OUTC", [128, 2 * 8 * XC], fp32,
                                       offset=_pbuf_off, align_bytes=32)
        PBUF3 = nc.alloc_sbuf_tensor_at("P3T", [128, 4096], fp16,
                                        offset=_g4_off, align_bytes=32)
        PBUF5 = nc.alloc_sbuf_tensor_at("P5T", [128, 4096], fp16,
                                        offset=_sqr_off, align_bytes=32)
        PBUF6 = nc.alloc_sbuf_tensor_at("P6T", [128, 4096], fp16,
                                        offset=_k25_off, align_bytes=32)
        ACC = es.enter_context(nc.psum_tensor("ACC", [128, 4096], fp32))

        v, s, g, t, sync = nc.vector, nc.scalar, nc.gpsimd, nc.tensor, nc.sync

        # ---- constants (emitted after the first guide loads; see below) ----
        def _emit_consts():
            sync.dma_start(out=RR[:], in_=rr_d[:])
            sync.dma_start(out=SIG[:], in_=sig_d[:])
            v.tensor_tensor(out=SIG2[:], in0=SIG[:], in1=SIG[:], op=ALU.mult)
            v.reciprocal(out=IS2[:], in_=SIG2[:])
            v.tensor_scalar(out=BROW[:], in0=RR[:], scalar1=IS2[0:1, 0:1],
                            scalar2=-0.5, op0=ALU.mult, op1=ALU.mult)
            wscr = nc.dram_tensor("wscr", [1, 25], fp32)
            sync.dma_start(out=wscr[:], in_=BROW[:])
            sync.dma_start(out=BROWT[:],
                           in_=dr_ap(wscr, 0, [[0, 128], [1, 25]]))

        # ---- fp16 staging casts (software-DGE d2d on gpsimd queue) ----
        GHD = nc.dram_tensor("GHD", [1, CG * GR * GXH + 2], fp16)
        g.dma_start(out=dr_ap(GHD, 1, [[1, CG * GR * GXH]]),
                    in_=dr_ap(guide_d, 0, [[1, CG * GR * GXH]]))
        INHD = nc.dram_tensor("INHD", [1, C * GR * IX + 2], fp16)
        _csz = (C // 4) * GR * IX
        for cc in range(4):
            g.dma_start(out=dr_ap(INHD, 1 + cc * _csz, [[1, _csz]]),
                        in_=dr_ap(inp_d, cc * _csz, [[1, _csz]]))

        # ---- guide slabs, dy0 pair first ----
        def g4_pair(dy):
            dst = sb(G4, 0, MR, dy * (CG * GX2), [[GX2, CG], [1, GX2]])
            src = dr_ap(GHD, 1 + dy * GXH + 3, [[GXH, MR], [GR * GXH, CG], [1, GX2]])
            sync.dma_start(out=dst, in_=src)
            dst = sb(G4O, 0, MR, dy * (CG * GX2), [[GX2, CG], [1, GX2]])
            src = dr_ap(GHD, 1 + dy * GXH + 4, [[GXH, MR], [GR * GXH, CG], [1, GX2]])
            sync.dma_start(out=dst, in_=src)

        DR_ORDER = [3, 4, 2, 5, 1, 6, 0]

        inb7_dmas = {}

        def inb7_load(j):
            dr = DR_ORDER[j]
            for wh in range(WH):
                dst = sb(INB7, wh * 64, 64, dr * XW, [[7 * XW, C], [1, XW]])
                src = dr_ap(INHD, 1 + dr * IX + wh * XC,
                            [[IX, 64], [GR * IX, C], [1, XW]])
                inb7_dmas.setdefault(j, []).append(
                    sync.dma_start(out=dst, in_=src))

        def odd_copy(j):
            # odd-parity (x-1) copy on ACT, idle once the exps are done
            dr = DR_ORDER[j]
            dst = sb(INB7O, 0, 128, dr * XW + 1, [[7 * XW, C], [1, XW - 1]])
            srcc = sb(INB7, 0, 128, dr * XW, [[7 * XW, C], [1, XW - 1]])
            s.copy(out=dst, in_=srcc)

        g4_pair(0)
        _emit_consts()
        inb7_load(0)
        for _dy in (1, 2, 3):
            g4_pair(_dy)
        sync.dma_start(out=IDENT[:], in_=ident_d[:])
        for _j in range(1, 7):
            inb7_load(_j)

        # ---- kA slots and remap groups (all issued during phase 1) ----
        ka_slot = {}
        for (uy, ux, m, shifted) in ALL_U:
            ka_slot[(uy, ux)] = m if not shifted else 24 + m
        AL_CHUNKS = {}
        CH = 4
        for m0c in range(0, 25, CH):
            nmap = min(CH, 25 - m0c)
            AL_CHUNKS.setdefault(m0c + nmap - 1, []).append((m0c, nmap))
        SH_GROUPS = {}
        _vy_base = {0: 1, 1: 4, 2: 11, 3: 18}
        for (vy, mb, nmap) in [(0, 1, 3), (1, 4, 4), (1, 8, 3), (2, 11, 4),
                               (2, 15, 3), (3, 18, 4), (3, 22, 3)]:
            vxmin = ((-3 if vy >= 1 else 1) + (mb - _vy_base[vy]))
            SH_GROUPS.setdefault(mb + nmap - 1, []).append((vy, mb, nmap, vxmin))

        # ---- phase 1: guide maps, DVE chain + ACT exp/remap-issue ----
        HOLD_MAP = 12
        last_exp = None
        for m, (uy, ux) in enumerate(UPLUS):
            if m == 0:
                g.memset(sb(K25, 0, MR, 0, [[1, MJ]]), 1.0)
            else:
                sl = m % NSLOT
                base = uy * (CG * GX2)
                if ux % 2 == 0:
                    in0 = sb(G4, 0, MR, base + AL0 + ux, [[GX2, CG], [1, MJ]])
                else:
                    in0 = sb(G4O, 0, MR, base + 3 + ux, [[GX2, CG], [1, MJ]])
                in1 = sb(G4, 0, MR, AL0, [[GX2, CG], [1, MJ]])
                v.tensor_tensor(out=sb(D3R, 0, MR, sl * CG * MJ,
                                       [[MJ, CG], [1, MJ]]),
                                in0=in0, in1=in1, op=ALU.subtract)
                v.tensor_tensor(out=sb(SQR, 0, MR, sl * CG * MJ, [[1, CG * MJ]]),
                                in0=sb(D3R, 0, MR, sl * CG * MJ, [[1, CG * MJ]]),
                                in1=sb(D3R, 0, MR, sl * CG * MJ, [[1, CG * MJ]]),
                                op=ALU.mult)
                v.tensor_tensor(out=sb(SRR, 0, MR, sl * SRW, [[1, MJ]]),
                                in0=sb(SQR, 0, MR, sl * CG * MJ, [[1, MJ]]),
                                in1=sb(SQR, 0, MR, sl * CG * MJ + MJ, [[1, MJ]]),
                                op=ALU.add)
                v.tensor_tensor(out=sb(SRR, 0, MR, sl * SRW + MJ, [[1, MJ]]),
                                in0=sb(SRR, 0, MR, sl * SRW, [[1, MJ]]),
                                in1=sb(SQR, 0, MR, sl * CG * MJ + 2 * MJ,
                                       [[1, MJ]]),
                                op=ALU.add)
                _e = s.activation(out=sb(K25, 0, MR, m * MS, [[1, MJ]]),
                                  in_=sb(SRR, 0, MR, sl * SRW + MJ,
                                         [[1, MJ]]),
                                  func=ACTF.Exp, scale=-0.5,
                                  bias=BROWT[0:MR, m:m + 1])
                if m == HOLD_MAP:
                    last_exp = _e
                if m == 20:
                    gate_exp = _e
            for (m0c, nmap) in AL_CHUNKS.get(m, ()):
                for wh in range(WH):
                    dst = sb(KA, wh * 64, 64, m0c * XC, [[XC, nmap], [1, XC]])
                    srcc = sb(K25, 3, 64, m0c * MS + wh * XC + 3,
                              [[MS, nmap], [1, XC]])
                    s.dma_start(out=dst, in_=srcc)
            for (vy, mb, nmap, vxmin) in SH_GROUPS.get(m, ()):
                for wh in range(WH):
                    dst = sb(KA, wh * 64, 64, (24 + mb) * XC,
                             [[XC, nmap], [1, XC]])
                    srcc = sb(K25, 3 - vy, 64,
                              mb * MS + wh * XC + 3 - vxmin,
                              [[MS - 1, nmap], [1, XC]])
                    s.dma_start(out=dst, in_=srcc)

        for _j in range(7):
            odd_copy(_j)

        # ---- phase 2: products ----
        evens = sorted((e for e in ALL_U if (3 + e[1]) % 2 == 0),
                       key=lambda e: e[2])
        odds = sorted((e for e in ALL_U if (3 + e[1]) % 2 == 1),
                      key=lambda e: e[2])
        order = evens + odds
        POOL_POS = {3, 9, 15, 21, 27, 33, 39, 45}
        n_mm = [0]
        ndve = [0]
        npool = [0]
        dve_since = [0]
        pend_pool = []
        dve_tts = []
        pool_tts = []

        def emit_mms(P):
            first = n_mm[0] == 0
            last = n_mm[0] == len(order) - 1
            for bk in range(8):
                t.matmul(ACC[:, bk * 512:(bk + 1) * 512], IDENT[:, :],
                         P[:, bk * 512:(bk + 1) * 512], start=first, stop=last)
            n_mm[0] += 1

        for oi, ent in enumerate(order):
            uy, ux, m, shifted = ent
            ui = ka_slot[(uy, ux)]
            off = (uy + 3) * XW + 3 + ux
            if (3 + ux) % 2 == 0:
                in0 = sb(INB7, 0, 128, off, [[7 * XW, C], [1, XC]])
            else:
                in0 = sb(INB7O, 0, 128, off + 1, [[7 * XW, C], [1, XC]])
            in1 = sb(KA, 0, 128, ui * XC, [[0, C], [1, XC]])
            if oi in POOL_POS:
                P = PBUF5 if npool[0] % 2 == 0 else PBUF6
                npool[0] += 1
                _pt = g.tensor_tensor(out=sb(P, 0, 128, 0, [[XC, C], [1, XC]]),
                                      in0=in0, in1=in1, op=ALU.mult)
                pool_tts.append(_pt)
                pend_pool.append(P)
            else:
                k = ndve[0]
                ndve[0] += 1
                cyc = [PBUFS[0], PBUFS[1], PBUFS[2], PBUF3]
                P = cyc[k % 4]
                _tt = v.tensor_tensor(out=sb(P, 0, 128, 0, [[XC, C], [1, XC]]),
                                      in0=in0, in1=in1, op=ALU.mult)
                dve_tts.append(_tt)
                if oi < 5 and last_exp is not None:
                    add_dep_helper(_tt.ins, last_exp.ins, sync=False,
                                   reason="hold products until guide done")
                emit_mms(P)
                dve_since[0] += 1
                if pend_pool and dve_since[0] >= 4:
                    emit_mms(pend_pool.pop(0))
                    dve_since[0] = 0
            # norm tree on gpsimd once its products are done
            if oi == 46:
                _nt = g.tensor_tensor(out=sb(T1, 0, 128, 0, [[1, 24 * XC]]),
                                in0=sb(KA, 0, 128, 0, [[1, 24 * XC]]),
                                in1=sb(KA, 0, 128, 24 * XC, [[1, 24 * XC]]),
                                op=ALU.add)
                add_dep_helper(_nt.ins, pool_tts[-1].ins, sync=False,
                               reason="norm tree after pool products")
                for nblk in (12, 6, 3):
                    g.tensor_tensor(out=sb(T1, 0, 128, 0, [[1, nblk * XC]]),
                                    in0=sb(T1, 0, 128, 0, [[1, nblk * XC]]),
                                    in1=sb(T1, 0, 128, nblk * XC,
                                           [[1, nblk * XC]]),
                                    op=ALU.add)
                g.tensor_tensor(out=sb(T1, 0, 128, 0, [[1, XC]]),
                                in0=sb(T1, 0, 128, 0, [[1, XC]]),
                                in1=sb(T1, 0, 128, XC, [[1, XC]]),
                                op=ALU.add)
                g.tensor_tensor(out=sb(T1, 0, 128, 0, [[1, XC]]),
                                in0=sb(T1, 0, 128, 0, [[1, XC]]),
                                in1=sb(T1, 0, 128, 2 * XC, [[1, XC]]),
                                op=ALU.add)
                g.tensor_tensor(out=sb(NORM, 0, 128, 0, [[1, XC]]),
                                in0=sb(T1, 0, 128, 0, [[1, XC]]),
                                in1=sb(KA, 0, 128, 48 * XC, [[1, XC]]),
                                op=ALU.add)
        while pend_pool:
            emit_mms(pend_pool.pop(0))
        _rc = v.reciprocal(out=RCP[:, :], in_=NORM[:, :])
        from concourse.tile import add_dep_helper as _adh
        _adh(_rc.ins, dve_tts[-2].ins, sync=False,
             reason="recip after products (scheduler hint)")

        # ---- finish: out = acc * rcp (bcast over c) ----
        for ch in range(4):
            obuf = (ch % 2) * 8 * XC
            a_sl = ACC[:, ch * 1024:(ch + 1) * 1024]
            a_ap = AP(a_sl.tensor, a_sl.offset, [a_sl.ap[0], [XC, 8], [1, XC]])
            r_ap = sb(RCP, 0, 128, 0, [[0, 8], [1, XC]])
            o_ap = sb(OUTC, 0, 128, obuf, [[XC, 8], [1, XC]])
            v.tensor_tensor(out=o_ap, in0=a_ap, in1=r_ap, op=ALU.mult)
            for wh in range(WH):
                srcc = sb(OUTC, wh * 64, 64, obuf, [[XC, 8], [1, XC]])
                dst = dr_ap(out_d, ch * 8 * RB * W + wh * XC,
                            [[W, 64], [RB * W, 8], [1, XC]])
                sync.dma_start(out=dst, in_=srcc)

    if legalize:
        _legalize_waits(nc)
    return nc


def _legalize_waits(nc):
    import concourse.mybir as mybir

    ctr = [0]
    for bb in nc.main_func.blocks:
        out = []
        changed = False
        for ins in bb.instructions:
            cap = 1
            si = ins.sync_info
            waits = list(si.on_wait) if si is not None else []
            if len(waits) > cap:
                keep = waits[:cap]
                extra = waits[cap:]
                while extra:
                    chunk, extra = extra[:1], extra[1:]
                    e = mybir.InstEventSemaphore(
                        name=f"wsplit-{ctr[0]}", ins=[], outs=[])
                    ctr[0] += 1
                    e.engine = ins.engine
                    e.sync_info = mybir.SyncInfo(on_wait=chunk, on_update=[])
                    out.append(e)
                ins.sync_info = mybir.SyncInfo(on_wait=keep,
                                               on_update=list(si.on_update))
                changed = True
            out.append(ins)
        if changed:
            bb.instructions = out
    return nc


def _host_prep(input, input_for_kernel, sigma_for_kernel):
    inp = np.asarray(input, dtype=np.float32)
    gui = np.asarray(input_for_kernel, dtype=np.float32)
    sig = np.float32(np.asarray(sigma_for_kernel).reshape(()))

    gp = np.zeros((B, CG, H + 12, W + 20), dtype=np.float32)
    gp[:, :, 6:6 + H, 10:10 + W] = gui
    ip = np.zeros((B, C, H + 12, W + 12), dtype=np.float32)
    ip[:, :, 6:6 + H, 6:6 + W] = inp

    rr = np.array([[float(uy * uy + ux * ux) for (uy, ux) in UPLUS]],
                  dtype=np.float32)
    ident = np.eye(128, dtype=np.float16)
    sig_arr = np.array([[sig]], dtype=np.float32)

    in_maps = []
    for core in range(NCORES):
        b, hb = divmod(core, NB)
        r0 = hb * RB
        gs = gp[b, :, 3 + r0: 3 + r0 + GR, 0:GXH]
        is_ = ip[b, :, 3 + r0: 3 + r0 + GR, 3:3 + IX]
        in_maps.append({
            "guide": np.ascontiguousarray(gs),
            "inp": np.ascontiguousarray(is_),
            "rr25": rr,
            "sigma": sig_arr,
            "ident": ident,
        })
    return in_maps


def kernel(input, input_for_kernel, sigma_for_kernel):
    global _COMPILED
    from concourse.bass_utils import run_bass_kernel_spmd

    if _COMPILED is None:
        _COMPILED = _build_nc()
    nc = _COMPILED

    in_maps = _host_prep(input, input_for_kernel, sigma_for_kernel)
    res = run_bass_kernel_spmd(nc, in_maps, core_ids=list(range(NCORES)))
    out = np.zeros((B, C, H, W), dtype=np.float32)
    for core in range(NCORES):
        b, hb = divmod(core, NB)
        out[b, :, hb * RB:(hb + 1) * RB, :] = res.results[core]["out"]
    return out
